# revision 40
# baseline (speedup 1.0000x reference)
"""DMGCN v4: cached edge features, LPT-balanced node tiles, deep pipelining.

Sharding: edges partitioned by dst core; within a core, nodes are relabeled
by LPT binning so every 128-node tile carries ~equal edge count (q flat at
~514), shrinking padded edge slots to ~1% over E/8. The subtile->window
scatter schedule (q profile) is compile-time and shared by all cores.

Structure per exec:
  prologue: h0 load (f16, SWDGE) + ev build (emb plane DMA + RBF on PE/ACT)
    stored to DRAM once, streamed back in every layer.
  per layer: pipelined edge loop with stage lag LAG so the PE never waits on
    same-tile cross-engine results: z-chain (fp8 DoubleRow z1; z2 skewed one
    tile) runs LAG tiles ahead of the consume stages (indirect-DMA gather
    hn[src] -> PE transpose -> product -> message matmul -> one-hot scatter
    into PSUM windows; one-hot S matrices are host-built and DMA-streamed).
    As dst windows close, the NEXT layer's node-MLP chunks (3-stage skewed
    pipeline) fire inside the loop, and the AllGather of hn is issued in 3
    pieces (src_row uses the piece-major cc_out layout) so only the last
    ~2304-row piece is exposed at the layer boundary. The final layer fires
    readout chunks instead; graph segment-sum on host.
"""
import os
import sys

for _p in ("/opt/trn_rl_repo", "/root/.axon_site/_ro/trn_rl_repo"):
    if os.path.isdir(_p) and _p not in sys.path:
        sys.path.insert(0, _p)

import numpy as np
import concourse.bass as bass
import concourse.mybir as mybir
import concourse.tile as tile
from concourse.bass_utils import run_bass_kernel_spmd
from concourse.masks import make_identity

N, E, G = 100000, 400000, 2000
D = 128
NC = 300
CUT_LO, CUT_HI = 0.0, 30.0
N_CONV = 3
NCORES = 8
P = 128
N_SH = N // NCORES            # 12500
NT = (N_SH + P - 1) // P      # 98
N_PAD = NT * P                # 12544
DE = 428

F32 = mybir.dt.float32
F16 = mybir.dt.float16
I32 = mybir.dt.int32
AF = mybir.ActivationFunctionType
ALU = mybir.AluOpType

PAD_OFF = 999.0
NP16 = "float16"
F8 = mybir.dt.float8e4

EG_GRP = 4            # 512-tiles per x3 load in the prologue
LAG = 20              # z-chain lead (tiles) over the consume stages
USE_2HOP = True       # 2-hop dma_gather (vs per-subtile indirect + PE transpose)
SEG = 4096            # edge slots per hn-gather epoch (8 tiles)
EP_TILES = SEG // 512


def wrap16(idx):
    """token i -> [i % 16, i // 16], replicated across the 8 Q7 cores."""
    n = len(idx)
    assert n % 16 == 0
    w = np.asarray(idx, np.int16).reshape(n // 16, 16).T
    return np.ascontiguousarray(np.tile(w, (8, 1)))


def split_waits(nc):
    """Walrus allows only 1 sync wait per instruction; hoist extras onto
    preceding NoOps on the same engine."""
    n_fix = 0
    for f in nc.m.functions:
        for blk in f.blocks:
            out = []
            for inst in blk.instructions:
                si = inst.sync_info
                if si and len(si.on_wait) > 1 and not isinstance(inst, mybir.InstNoOp):
                    waits = list(si.on_wait)
                    for w in waits[:-1]:
                        nop = mybir.InstNoOp(name=f"{inst.name}-ws{n_fix}", ins=[], outs=[])
                        nop.engine = inst.engine
                        nop.sync_info = mybir.SyncInfo(on_wait=[w], on_update=[])
                        out.append(nop)
                        n_fix += 1
                    si.on_wait = [waits[-1]]
                out.append(inst)
            blk.instructions[:] = out
    return n_fix


def finalize_v2(nc):
    """finalize + ucode library loads + extended-inst codegen + wait splits."""
    nc.finalize()
    import bass_rust
    from concourse.library_config import all_libraries, standard
    from concourse.library_overlay import lower_extended_insts
    mask = {}
    for lib in all_libraries:
        for it in lib.instructions:
            mask[it] = mask.get(it, 0) | (1 << lib.index)
    bass_rust.insert_library_loads(nc, mask, len(all_libraries), standard.index)
    lower_extended_insts(nc)
    split_waits(nc)
    return nc


def geometry(q):
    """Compile-time maps from the uniform slot profile q[98]."""
    q = np.asarray(q, np.int64)
    Q = np.concatenate([[0], np.cumsum(q)])
    slots = int(Q[NT])
    e_slots = ((slots + 511) // 512) * 512
    n_sub = e_slots // 128
    n512 = e_slots // 512
    tiles_of, w1 = [], []
    for s in range(n_sub):
        lo, hi = 128 * s, 128 * (s + 1)
        ts = [t for t in range(NT) if Q[t] < hi and Q[t + 1] > lo]
        assert len(ts) <= 2, (s, ts)
        tiles_of.append(ts)
        w1.append(ts[0] if ts else -1)
    s_first = {t: int(Q[t] // 128) for t in range(NT)}
    s_last = {t: int((Q[t + 1] - 1) // 128) for t in range(NT)}
    return Q, e_slots, n_sub, n512, tiles_of, w1, s_first, s_last


def host_prep(inputs):
    Z = np.asarray(inputs["Z"]).astype(np.int64)
    edge_type = np.asarray(inputs["edge_type"]).astype(np.int64)
    dist = np.asarray(inputs["dist"]).astype(np.float32)
    src = np.asarray(inputs["src"]).astype(np.int64)
    dst = np.asarray(inputs["dst"]).astype(np.int64)
    graph_ids = np.asarray(inputs["graph_ids"]).astype(np.int64)

    # LPT-balance nodes into 98 bins of <=128 per core (flattens the
    # per-tile slot profile q); newpos maps node -> relabeled position
    deg_all = np.bincount(dst, minlength=N)
    newpos = np.zeros(N, np.int64)
    inv_pos = np.zeros((NCORES, N_PAD), np.int64)    # position -> local node
    for c in range(NCORES):
        degc = deg_all[c * N_SH:(c + 1) * N_SH]
        order_c = np.argsort(-degc, kind="stable")
        import heapq
        heap = [(0, t) for t in range(NT)]
        heapq.heapify(heap)
        fill = np.zeros(NT, np.int64)
        binw = np.zeros(NT, np.int64)
        bin_of = np.zeros(N_SH, np.int64)
        for v in order_c:
            w_, t = heapq.heappop(heap)
            bin_of[v] = t
            binw[t] += degc[v]
            fill[t] += 1
            if fill[t] < P:
                heapq.heappush(heap, (binw[t], t))
        # rank bins by weight desc so tile t pairs heavy-with-heavy across cores
        rank_of = np.zeros(NT, np.int64)
        rank_of[np.argsort(-binw, kind="stable")] = np.arange(NT)
        slot_ctr = np.zeros(NT, np.int64)
        for v in range(N_SH):
            t = rank_of[bin_of[v]]
            p_ = slot_ctr[t]
            slot_ctr[t] += 1
            newpos[c * N_SH + v] = c * N_PAD + t * P + p_
            inv_pos[c, t * P + p_] = v
    dkey = newpos[dst]
    order = np.argsort(dkey, kind="stable")
    dsts = dkey[order]
    srcs = src[order]
    dists = dist[order]
    etypes = edge_type[order]

    core_lo = np.searchsorted(dsts, np.arange(NCORES) * N_PAD, side="left")
    core_hi = np.append(core_lo[1:], E)

    cnt = np.zeros((NCORES, NT), dtype=np.int64)
    for c in range(NCORES):
        dl = dsts[core_lo[c]:core_hi[c]] - c * N_PAD
        cnt[c] = np.bincount(dl // P, minlength=NT)
    q = cnt.max(axis=0)
    Q, e_slots, n_sub, n512, tiles_of, w1, s_first, s_last = geometry(q)

    HB = (0, 6144, 10240, N_PAD)             # cc piece boundaries
    npc = newpos[srcs] // N_PAD              # core of src
    npp = newpos[srcs] % N_PAD               # position within core
    src_row = np.zeros(E, np.int64)
    base = 0
    for k in range(3):
        lo_, hi_ = HB[k], HB[k + 1]
        m_ = (npp >= lo_) & (npp < hi_)
        src_row[m_] = base + npc[m_] * (hi_ - lo_) + (npp[m_] - lo_)
        base += NCORES * (hi_ - lo_)
    src_row = src_row.astype(np.int32)

    def to_pf(arr, width):
        return np.ascontiguousarray(arr.reshape(width, P).T)

    import ml_dtypes
    core_in = []
    for c in range(NCORES):
        lo = core_lo[c]
        sr = np.zeros(e_slots, dtype=np.int32)
        doff = np.full(e_slots, PAD_OFF, dtype=np.float32)
        dd = np.zeros(e_slots, dtype=np.float32)
        et = np.zeros(e_slots, dtype=np.int64)
        start = 0
        for t in range(NT):
            n_ct = int(cnt[c, t])
            base = int(Q[t])
            sl = slice(lo + start, lo + start + n_ct)
            ks = np.arange(base, base + n_ct)
            sr[ks] = src_row[sl]
            reb = 128 * np.asarray([w1[k // 128] for k in ks], np.int64)
            dloc = dsts[sl] - c * N_PAD
            dv = (dloc - reb).astype(np.float32)
            assert (dv >= 0).all() and (dv < 256).all()
            doff[ks] = dv
            dd[ks] = dists[sl]
            et[ks] = etypes[sl]
            start += n_ct
        assert start == core_hi[c] - core_lo[c]
        x3 = np.stack([dd, dd * dd, np.ones_like(dd)], 0).astype(np.float32)
        Zc = Z[c * N_SH + inv_pos[c]]          # position-order Z (pads dup)
        efm8 = np.ascontiguousarray(
            np.asarray(inputs["edge_emb"])[et].T.astype(ml_dtypes.float8_e4m3))
        h0f = np.ascontiguousarray(
            np.asarray(inputs["node_emb"])[Zc].T.astype(NP16))
        core_in.append(dict(
            src_row=to_pf(sr, n_sub),
            dst_off=to_pf(doff, n_sub).astype(np.float16), x3=x3,
            efm8=efm8, h0f=h0f,
        ))

    w = {}
    centers = np.linspace(CUT_LO, CUT_HI, NC, dtype=np.float32)
    gap = np.float32(centers[1] - centers[0])
    A = np.stack([2.0 * centers / gap,
                  -np.ones(NC, np.float32) / gap,
                  -(centers ** 2) / gap], 0).astype(np.float32)
    w["A"] = np.concatenate([A, np.zeros((3, 384 - NC), np.float32)], 1)
    for i in range(N_CONV):
        w[f"wn1t_{i}"] = np.ascontiguousarray(np.asarray(inputs["Wn1"][i]).T.astype(NP16))
        w[f"wn2t_{i}"] = np.ascontiguousarray(np.asarray(inputs["Wn2"][i]).T.astype(NP16))
        f8 = ml_dtypes.float8_e4m3
        we1t = np.zeros((512, 512), np.float32)      # K-pad 428->512, M-pad 428->512
        we1t[:DE, :DE] = np.asarray(inputs["We1"][i]).T
        we2t = np.zeros((512, D), np.float32)
        we2t[:DE, :] = np.asarray(inputs["We2"][i]).T
        for m in range(4):
            mc = we1t[:, m * 128:(m + 1) * 128]
            w[f"wdr_{i}_{m}_0"] = np.ascontiguousarray(
                np.stack([mc[0:128], mc[128:256]], 1).astype(f8))    # [128,2,128]
            w[f"wdr_{i}_{m}_1"] = np.ascontiguousarray(
                np.stack([mc[256:384], mc[384:512]], 1).astype(f8))
        w[f"w2dr_{i}_0"] = np.ascontiguousarray(
            np.stack([we2t[0:128], we2t[128:256]], 1).astype(f8))
        w[f"w2dr_{i}_1"] = np.ascontiguousarray(
            np.stack([we2t[256:384], we2t[384:512]], 1).astype(f8))
        w[f"wct_{i}"] = np.ascontiguousarray(np.asarray(inputs["Wc"][i]).T.astype(NP16))
        w[f"bn1_{i}"] = np.asarray(inputs["bn1"][i]).reshape(D, 1).astype(np.float32)
        w[f"bn2_{i}"] = np.asarray(inputs["bn2"][i]).reshape(D, 1).astype(np.float32)
        w[f"be1_{i}"] = np.asarray(inputs["be1"][i]).reshape(DE, 1).astype(np.float32)
        w[f"be2_{i}"] = np.asarray(inputs["be2"][i]).reshape(D, 1).astype(np.float32)
        w[f"bc_{i}"] = np.ascontiguousarray(
            np.tile(np.asarray(inputs["bc"][i]).reshape(1, D), (1, 4))).astype(NP16)
    w["wr1t"] = np.ascontiguousarray(np.asarray(inputs["Wr1"]).T.astype(NP16))
    w["wr2t"] = np.ascontiguousarray(np.asarray(inputs["Wr2"]).T.astype(NP16))
    w["br1"] = np.asarray(inputs["br1"]).reshape(D, 1).astype(np.float32)
    w["br2"] = np.full((D, 1), np.asarray(inputs["br2"]).reshape(()),
                       dtype=np.float32)

    # ---- 2-hop hn gather: epochs of SEG slots, 4 src-range chunks ----
    n_ep = (e_slots + SEG - 1) // SEG
    ck_all = []
    for c in range(NCORES):
        sr_flat = core_in[c]["src_row"].T.reshape(-1)      # slot order
        ck_all.append((sr_flat.astype(np.int64) // 32768).astype(np.int64))
    cnt_ec = np.zeros((NCORES, n_ep, 4), np.int64)
    for c in range(NCORES):
        for e in range(n_ep):
            seg = ck_all[c][e * SEG:(e + 1) * SEG]
            cnt_ec[c, e] = np.bincount(seg, minlength=4)
    P_ec = ((cnt_ec.max(axis=0) + 127) // 128) * 128       # [n_ep, 4]
    segb = np.zeros((n_ep, 5), np.int64)
    segb[:, 1:] = np.cumsum(P_ec, axis=1)
    T_MAX = int(segb[:, 4].max())
    S_tot = T_MAX // 128
    h1b = np.zeros(n_ep * 4 + 1, np.int64)
    h1b[1:] = np.cumsum(P_ec.reshape(-1))
    h1tot = int(h1b[-1])
    for c in range(NCORES):
        sr_flat = core_in[c]["src_row"].T.reshape(-1)
        ck = ck_all[c]
        h1 = np.zeros(h1tot, np.int16)
        h2 = np.zeros(e_slots, np.int64)
        for e in range(n_ep):
            lo_, hi_ = e * SEG, min((e + 1) * SEG, e_slots)
            seg_sr = sr_flat[lo_:hi_]
            seg_ck = ck[lo_:hi_]
            for k in range(4):
                sel = np.nonzero(seg_ck == k)[0]
                base = int(h1b[e * 4 + k])
                h1[base:base + len(sel)] = (seg_sr[sel] - 32768 * k).astype(np.int16)
                # staging DMA writes stage[p, s] (hop1 index i = s*128+p) to
                # DRAM row p*S_tot + s (p-major pairing, HW-verified)
                iv = segb[e, k] + np.arange(len(sel))
                h2[lo_ + sel] = (iv % 128) * S_tot + iv // 128
        core_in[c]["h1"] = wrap16(h1)
        core_in[c]["h2"] = wrap16(h2)

    meta = (tuple(int(x) for x in q),
            tuple(tuple(int(x) for x in row) for row in P_ec))
    return core_in, w, meta, (graph_ids, newpos)


KCH = [(0, 128), (128, 256), (256, 384), (384, 428)]


def build_nc(meta, reps=1):
    q, P_ec_t = meta
    P_ec = np.asarray(P_ec_t, np.int64)
    n_ep = P_ec.shape[0]
    segb = np.zeros((n_ep, 5), np.int64)
    segb[:, 1:] = np.cumsum(P_ec, axis=1)
    T_MAX = int(segb[:, 4].max())
    h1b = np.zeros(n_ep * 4 + 1, np.int64)
    h1b[1:] = np.cumsum(P_ec.reshape(-1))
    h1tot = int(h1b[-1])
    Q, e_slots, n_sub, n512, tiles_of, w1, s_first, s_last = geometry(q)

    nc = bass.Bass(num_devices=NCORES, num_swdge_queues=2)

    t_in = {}

    def inp(name, shp, dt=F32):
        t_in[name] = nc.dram_tensor(name, shp, dt, kind="ExternalInput")
        return t_in[name]

    src_row = inp("src_row", [P, n_sub], I32)
    dst_off = inp("dst_off", [P, n_sub], F16)
    x3 = inp("x3", [3, e_slots], F32)
    efm8_t = inp("efm8", [P, e_slots], F8)
    h0f_t = inp("h0f", [P, N_PAD], F16)
    A_t = inp("A", [3, 384], F32)
    I16 = mybir.dt.int16
    h1_t = inp("h1", [P, h1tot // 16], I16)
    h2_t = inp("h2", [P, e_slots // 16], I16)
    for i in range(N_CONV):
        for nm, shp in (("bn1", [D, 1]), ("bn2", [D, 1]), ("be1", [DE, 1]),
                        ("be2", [D, 1])):
            inp(f"{nm}_{i}", shp)
        for nm, shp in (("wn1t", [D, D]), ("wn2t", [D, D]),
                        ("wct", [D, D]), ("bc", [1, 4 * D])):
            inp(f"{nm}_{i}", shp, F16)
        for m in range(4):
            inp(f"wdr_{i}_{m}_0", [P, 2, P], F8)
            inp(f"wdr_{i}_{m}_1", [P, 2, P], F8)
        inp(f"w2dr_{i}_0", [P, 2, P], F8)
        inp(f"w2dr_{i}_1", [P, 2, P], F8)
    inp("wr1t", [D, D], F16); inp("wr2t", [D, 1], F16)
    inp("br1", [D, 1]); inp("br2", [D, 1])
    r_out = nc.dram_tensor("r_out", [1, N_PAD], F32, kind="ExternalOutput")

    cc_in = [nc.dram_tensor(f"cc_in_{i}", [N_PAD, D], F16, kind="Internal")
             for i in range(N_CONV)]
    ev_d = nc.dram_tensor("ev_d", [P, n512, 4, 512], F8, kind="Internal")
    S_tot = T_MAX // 128
    n_ep_b = (e_slots + SEG - 1) // SEG
    tbl_d = nc.dram_tensor("tbl_d", [n_ep_b * P * S_tot, D], F16, kind="Internal")
    cc_out = [nc.dram_tensor(f"cc_out_{i}", [NCORES * N_PAD, D], F16,
                             kind="Internal", addr_space="Shared")
              for i in range(N_CONV)]

    with tile.TileContext(nc) as tc:
        with (
            tc.tile_pool(name="const", bufs=1) as cp,
            tc.tile_pool(name="sb", bufs=3) as sb,
            tc.tile_pool(name="x3p", bufs=3) as xp,
            tc.tile_pool(name="tbp", bufs=2) as tbp,
            tc.tile_pool(name="hbp", bufs=(LAG + 3) // 4 + 2) as hp,
            tc.tile_pool(name="evp", bufs=4) as ep,
            tc.tile_pool(name="ps", bufs=1, space="PSUM") as ps,
        ):
            ident = cp.tile([P, P], F32)
            make_identity(nc, ident[:])
            ident16 = cp.tile([P, P], F16)
            nc.vector.tensor_copy(out=ident16[:], in_=ident[:])
            iota_i = cp.tile([P, P], I32)
            nc.gpsimd.iota(iota_i[:], pattern=[[1, P]], base=0, channel_multiplier=0)
            iota_lo = cp.tile([P, P], F16)
            nc.vector.tensor_copy(out=iota_lo[:], in_=iota_i[:])
            iota_hi = cp.tile([P, P], F16)
            nc.vector.tensor_scalar(out=iota_hi[:], in0=iota_lo[:],
                                    scalar1=128.0, scalar2=None, op0=ALU.add)
            ones_row = cp.tile([1, P], F16)
            nc.vector.memset(ones_row[:], 1.0)

            def load_const(name, shp, dt=F32):
                tl = cp.tile(shp, dt, tag=name)
                nc.sync.dma_start(out=tl[:], in_=t_in[name][:, :])
                return tl

            A_sb = load_const("A", [3, 384])
            wr1t_sb = load_const("wr1t", [D, D], F16)
            wr2t_sb = load_const("wr2t", [D, 1], F16)
            br1_sb = load_const("br1", [D, 1])
            br2_sb = load_const("br2", [D, 1])
            W = {}
            for i in range(N_CONV):
                for nm in ("bn1", "bn2", "be2"):
                    W[f"{nm}_{i}"] = load_const(f"{nm}_{i}", [D, 1])
                for nm, shp in (("wn1t", [D, D]), ("wn2t", [D, D]),
                                ("wct", [D, D]), ("bc", [1, 4 * D])):
                    W[f"{nm}_{i}"] = load_const(f"{nm}_{i}", shp, F16)
                for m in range(4):
                    for pp in range(2):
                        tl = cp.tile([P, 2, P], F8, tag=f"wdr_{i}_{m}_{pp}")
                        nc.sync.dma_start(out=tl[:], in_=t_in[f"wdr_{i}_{m}_{pp}"][:, :, :])
                        W[f"wdr_{i}_{m}_{pp}"] = tl
                for pp in range(2):
                    tl = cp.tile([P, 2, P], F8, tag=f"w2dr_{i}_{pp}")
                    nc.sync.dma_start(out=tl[:], in_=t_in[f"w2dr_{i}_{pp}"][:, :, :])
                    W[f"w2dr_{i}_{pp}"] = tl
                be1 = cp.tile([P, 4], F32, tag=f"be1_{i}")
                nc.vector.memset(be1[:], 0.0)
                for k, (k0, k1) in enumerate(KCH):
                    nc.sync.dma_start(out=be1[:k1 - k0, k:k + 1],
                                      in_=t_in[f"be1_{i}"][k0:k1, :])
                W[f"be1_{i}"] = be1

            h_fm = cp.tile([P, N_PAD], F32, tag="h_fm")

            sri = cp.tile([P, n_sub], I32, tag="sri_sb")
            nc.sync.dma_start(out=sri[:], in_=src_row[:, :])
            h1_sb = cp.tile([P, h1tot // 16], I16, tag="h1_sb")
            nc.sync.dma_start(out=h1_sb[:], in_=h1_t[:, :])
            h2_sb = cp.tile([P, e_slots // 16], I16, tag="h2_sb")
            nc.sync.dma_start(out=h2_sb[:], in_=h2_t[:, :])

            _regs = {}

            def nreg(v):
                if v not in _regs:
                    _regs[v] = nc.gpsimd.to_reg(v)
                return _regs[v]

            for rep_ in range(reps):
                # ---------- prologue: h0 (host-precomputed, SWDGE queue) ----
                for a0 in range(0, N_PAD, 2048):
                    nn = min(2048, N_PAD - a0)
                    nc.gpsimd.dma_start(out=h_fm[:, a0:a0 + nn],
                                        in_=h0f_t[:, a0:a0 + nn])

                nm_state = {}

                def nmA(i, c):
                    j0 = 512 * c
                    wdt = min(512, N_PAD - j0)
                    h16 = sb.tile([P, 512], F16, tag="h16")
                    nc.vector.tensor_copy(out=h16[:, :wdt], in_=h_fm[:, j0:j0 + wdt])
                    ps1 = ps.tile([P, 512], F32, space="PSUM", tag="nm",
                                  bufs=1)
                    nc.tensor.matmul(out=ps1[:, :wdt], lhsT=W[f"wn1t_{i}"][:],
                                     rhs=h16[:, :wdt], start=True, stop=True)
                    zb = sb.tile([P, 512], F16, tag="nmlp_z")
                    nc.scalar.activation(out=zb[:, :wdt], in_=ps1[:, :wdt],
                                         func=AF.Relu, bias=W[f"bn1_{i}"][:, :1])
                    nm_state[(i, c, 'A')] = zb

                def nmB(i, c):
                    j0 = 512 * c
                    wdt = min(512, N_PAD - j0)
                    zb = nm_state.pop((i, c, 'A'))
                    ps2 = ps.tile([P, 512], F32, space="PSUM", tag="nm",
                                  bufs=1)
                    nc.tensor.matmul(out=ps2[:, :wdt], lhsT=W[f"wn2t_{i}"][:],
                                     rhs=zb[:, :wdt], start=True, stop=True)
                    hnb = sb.tile([P, 512], F16, tag="nmlp_hn")
                    nc.scalar.activation(out=hnb[:, :wdt], in_=ps2[:, :wdt],
                                         func=AF.Identity, bias=W[f"bn2_{i}"][:, :1])
                    nm_state[(i, c, 'B')] = hnb

                def nmC(i, c):
                    j0 = 512 * c
                    wdt = min(512, N_PAD - j0)
                    hnb = nm_state.pop((i, c, 'B'))
                    ptw = ps.tile([P, 512], F16, space="PSUM", tag="px",
                                  bufs=2)
                    for a in range(wdt // P):
                        nc.tensor.transpose(out=ptw[:, a * P:(a + 1) * P],
                                            in_=hnb[:, a * P:(a + 1) * P],
                                            identity=ident16[:])
                    hnm = sb.tile([P, 512], F16, tag="hn_nm")
                    nc.vector.tensor_copy(out=hnm[:, :wdt], in_=ptw[:, :wdt])
                    for a in range(wdt // P):
                        nc.sync.dma_start(
                            out=cc_in[i][j0 + a * P:j0 + (a + 1) * P, :],
                            in_=hnm[:, a * P:(a + 1) * P])

                def roA(c):
                    j0 = 512 * c
                    wdt = min(512, N_PAD - j0)
                    h16r = sb.tile([P, 512], F16, tag="h16")
                    nc.vector.tensor_copy(out=h16r[:, :wdt], in_=h_fm[:, j0:j0 + wdt])
                    ps1 = ps.tile([P, 512], F32, space="PSUM", tag="nm", bufs=1)
                    nc.tensor.matmul(out=ps1[:, :wdt], lhsT=wr1t_sb[:],
                                     rhs=h16r[:, :wdt], start=True, stop=True)
                    qb = sb.tile([P, 512], F16, tag="qb")
                    nc.scalar.activation(out=qb[:, :wdt], in_=ps1[:, :wdt],
                                         func=AF.Relu, bias=br1_sb[:, :1])
                    nm_state[(99, c, 'A')] = qb

                def roB(c):
                    j0 = 512 * c
                    wdt = min(512, N_PAD - j0)
                    qb = nm_state.pop((99, c, 'A'))
                    prt = ps.tile([P, 512], F32, space="PSUM", tag="nm", bufs=1)
                    nc.tensor.matmul(out=prt[:1, :wdt], lhsT=wr2t_sb[:],
                                     rhs=qb[:, :wdt], start=True, stop=True)
                    rsb = sb.tile([1, 512], F32, tag="rsb")
                    nc.scalar.activation(out=rsb[:, :wdt], in_=prt[:1, :wdt],
                                         func=AF.Identity, bias=br2_sb[:1, :1])
                    nc.sync.dma_start(out=r_out[:, j0:j0 + wdt],
                                      in_=rsb[:, :wdt])

                HB = (0, 6144, 10240, N_PAD)
                CC_AT = {HB[1] // 512 - 1: 0, HB[2] // 512 - 1: 1}

                def issue_cc(i, piece):
                    lo_, hi_ = HB[piece], HB[piece + 1]
                    base = NCORES * HB[piece]
                    nc.gpsimd.collective_compute(
                        "AllGather", ALU.bypass,
                        replica_groups=[list(range(NCORES))],
                        ins=[cc_in[i][lo_:hi_, :]],
                        outs=[cc_out[i][base:base + NCORES * (hi_ - lo_), :]])

                N_CHUNK = (N_PAD + 511) // 512

                def ev_tile(j):
                    js = slice(j * 512, (j + 1) * 512)
                    x3q = xp.tile([3, 512], F32, tag="x3q")
                    nc.sync.dma_start(out=x3q[:], in_=x3[:, js])
                    evt = ep.tile([P, 4, 512], F8, tag="evb")
                    nc.sync.dma_start(out=evt[:, 0, :], in_=efm8_t[:, js])
                    for k in range(3):
                        pvt = ps.tile([P, 512], F32, space="PSUM", tag="px",
                                      bufs=2)
                        nc.tensor.matmul(out=pvt[:],
                                         lhsT=A_sb[:, k * 128:(k + 1) * 128],
                                         rhs=x3q[:], start=True, stop=True)
                        nc.scalar.activation(out=evt[:, k + 1, :], in_=pvt[:],
                                             func=AF.Exp)
                    nc.sync.dma_start(out=ev_d[:, j, :, :], in_=evt[:])

                EV_PRE = min(6, n512)
                for j in range(EV_PRE):
                    ev_tile(j)

                # ---------- layer 0 node MLP + collective (2-skewed) ------
                for c in range(N_CHUNK + 2):
                    if c < N_CHUNK:
                        nmA(0, c)
                    if 1 <= c <= N_CHUNK:
                        nmB(0, c - 1)
                    if 2 <= c:
                        nmC(0, c - 2)
                        if c - 2 in CC_AT:
                            issue_cc(0, CC_AT[c - 2])
                issue_cc(0, 2)

                # ---------- prologue: ev planes -> DRAM ----------
                for j in range(EV_PRE, n512):
                    ev_tile(j)

                # ---------- layers ----------
                for i in range(N_CONV):
                    # --- pipelined edge loop ---
                    eet_t = {}
                    prod_t = {}
                    msb_t = {}
                    tbl_t = {}
                    hb_t = {}
                    S_t = {}
                    pd_map = {}
                    pend_close = []

                    def hop1(e):
                        # gather this epoch's hn rows (4 src ranges) into SBUF,
                        # then stage to DRAM in one DMA (p-major row layout)
                        tbl = tbp.tile([P, S_tot, P], F16, tag="hntbl")
                        for k in range(4):
                            lo_r = 32768 * k
                            hi_r = min(32768 * (k + 1), NCORES * N_PAD)
                            pk = int(P_ec[e, k])
                            for off in range(0, pk, 1024):
                                nn = min(1024, pk - off)
                                ob = int(segb[e, k]) + off
                                ib = int(h1b[e * 4 + k]) + off
                                nc.gpsimd.dma_gather(
                                    out_ap=tbl[:, ob // 128:(ob + nn) // 128, :],
                                    in_ap=cc_out[i][lo_r:hi_r, :],
                                    idxs_ap=h1_sb[:, ib // 16:(ib + nn) // 16],
                                    num_idxs=nn, num_idxs_reg=nreg(nn),
                                    elem_size=D, transpose=False)
                        nc.sync.dma_start(
                            out=tbl_d[e * P * S_tot:(e + 1) * P * S_tot, :],
                            in_=tbl[:])

                    def hop2(j):
                        # slot-order hn columns for tile j from the DRAM table
                        e = j // EP_TILES
                        hb = hp.tile([P, 1, 512], F16, tag="hb", bufs=LAG + 3)
                        nc.gpsimd.dma_gather(
                            out_ap=hb[:], in_ap=tbl_d[e * P * S_tot:
                                                      (e + 1) * P * S_tot, :],
                            idxs_ap=h2_sb[:, j * 32:(j + 1) * 32],
                            num_idxs=512, num_idxs_reg=nreg(512),
                            elem_size=D, transpose=True, queue_num=1)
                        hb_t[j] = hb

                    ev_t = {}

                    def load_ev(j):
                        evt = ep.tile([P, 4, 512], F8, tag="evb")
                        nc.sync.dma_start(out=evt[:], in_=ev_d[:, j, :, :])
                        ev_t[j] = evt

                    z1p_t = {}

                    def z_stage(j):
                        evt = ev_t.pop(j)
                        z1pa = sb.tile([P, 2, 512], F8, tag="z1p0", bufs=4)
                        z1pb = sb.tile([P, 2, 512], F8, tag="z1p1", bufs=4)
                        z1p = [z1pa, z1pb]
                        for m in range(4):
                            pz = ps.tile([P, 512], F32, space="PSUM", tag="z1",
                                         bufs=2)
                            nc.tensor.matmul(out=pz[:], lhsT=W[f"wdr_{i}_{m}_0"][:],
                                             rhs=evt[:, 0:2, :], start=True, stop=False,
                                             perf_mode=mybir.MatmulPerfMode.DoubleRow)
                            nc.tensor.matmul(out=pz[:], lhsT=W[f"wdr_{i}_{m}_1"][:],
                                             rhs=evt[:, 2:4, :], start=False, stop=True,
                                             perf_mode=mybir.MatmulPerfMode.DoubleRow)
                            if m < 2:
                                nc.vector.tensor_scalar(
                                    out=z1p[m // 2][:, m % 2, :], in0=pz[:],
                                    scalar1=W[f"be1_{i}"][:, m:m + 1],
                                    scalar2=0.0, op0=ALU.add, op1=ALU.max)
                            else:
                                nc.scalar.activation(
                                    out=z1p[m // 2][:, m % 2, :], in_=pz[:],
                                    func=AF.Relu,
                                    bias=W[f"be1_{i}"][:, m:m + 1])
                        z1p_t[j] = (z1pa, z1pb)

                    def z2_stage(j):
                        z1pa, z1pb = z1p_t.pop(j)
                        pe = ps.tile([P, 512], F32, space="PSUM", tag="z2",
                                     bufs=1)
                        nc.tensor.matmul(out=pe[:], lhsT=W[f"w2dr_{i}_0"][:],
                                         rhs=z1pa[:], start=True, stop=False,
                                         perf_mode=mybir.MatmulPerfMode.DoubleRow)
                        nc.tensor.matmul(out=pe[:], lhsT=W[f"w2dr_{i}_1"][:],
                                         rhs=z1pb[:], start=False, stop=True,
                                         perf_mode=mybir.MatmulPerfMode.DoubleRow)
                        eet = sb.tile([P, 512], F16, tag="eet", bufs=LAG + 3)
                        nc.scalar.activation(out=eet[:], in_=pe[:], func=AF.Identity,
                                             bias=W[f"be2_{i}"][:, :1])
                        eet_t[j] = eet

                    gath_t = {}

                    def issue_gather(j):
                        gs = []
                        for a in range(4):
                            s_ = 4 * j + a
                            g = hp.tile([P, D], F16, tag="gath",
                                        bufs=4 * (LAG + 3))
                            nc.gpsimd.indirect_dma_start(
                                out=g[:], out_offset=None, in_=cc_out[i][:, :],
                                in_offset=bass.IndirectOffsetOnAxis(
                                    ap=sri[:, s_:s_ + 1], axis=0))
                            gs.append(g)
                        gath_t[j] = gs

                    def p_stage_ind(j):
                        ptw = ps.tile([P, 512], F16, space="PSUM", tag="px",
                                      bufs=2)
                        g = gath_t.pop(j)
                        for a in range(4):
                            nc.tensor.transpose(out=ptw[:, a * P:(a + 1) * P],
                                                in_=g[a][:], identity=ident16[:])
                        prod = sb.tile([P, 512], F16, tag="prod")
                        nc.vector.tensor_mul(out=prod[:], in0=eet_t.pop(j)[:],
                                             in1=ptw[:])
                        prod_t[j] = prod

                    def p_stage(j):
                        if not USE_2HOP:
                            p_stage_ind(j)
                            return
                        hb = hb_t.pop(j)
                        prod = sb.tile([P, 512], F16, tag="prod")
                        nc.vector.tensor_mul(out=prod[:], in0=eet_t.pop(j)[:],
                                             in1=hb[:, 0, :])
                        prod_t[j] = prod

                    def m_stage(j):
                        pmw = ps.tile([P, 512], F32, space="PSUM", tag="px",
                                      bufs=2)
                        nc.tensor.matmul(out=pmw[:], lhsT=ones_row[:],
                                         rhs=W[f"bc_{i}"][:, :], start=True, stop=False)
                        prod = prod_t.pop(j)
                        for a in range(4):
                            nc.tensor.matmul(out=pmw[:, a * P:(a + 1) * P],
                                             lhsT=prod[:, a * P:(a + 1) * P],
                                             rhs=W[f"wct_{i}"][:], start=False,
                                             stop=True, skip_group_check=True)
                        msb = sb.tile([P, 512], F16, tag="msb")
                        nc.scalar.activation(out=msb[:], in_=pmw[:], func=AF.Tanh)
                        msb_t[j] = msb

                    def s_prep(j):
                        # one-hot columns for tile j, built one iteration early
                        for a in range(4):
                            s = 4 * j + a
                            for t in tiles_of[s]:
                                wo = t - w1[s]
                                S = sb.tile([P, P], F16, tag="S", bufs=12)
                                nc.vector.tensor_tensor(
                                    out=S[:],
                                    in0=doff_sb[:, s:s + 1].to_broadcast([P, P]),
                                    in1=(iota_lo if wo == 0 else iota_hi)[:],
                                    op=ALU.is_equal)
                                S_t[(s, t)] = S

                    closed_hi = [0]
                    nxt_chunk = [0]
                    qA, qB, qC = [], [], []

                    def fire_chunks():
                        # queue next-layer node MLP (or final readout) chunks
                        # whose h_fm windows are final
                        while (nxt_chunk[0] < N_CHUNK
                               and closed_hi[0] >= min(4 * (nxt_chunk[0] + 1), NT)):
                            qA.append(nxt_chunk[0])
                            nxt_chunk[0] += 1

                    def pump_chunks():
                        # advance the staged chunk pipeline, one per stage
                        if qC:
                            c = qC.pop(0)
                            if i + 1 < N_CONV:
                                nmC(i + 1, c)
                                if c in CC_AT:
                                    issue_cc(i + 1, CC_AT[c])
                            else:
                                roB(c)
                        if qB:
                            c = qB.pop(0)
                            if i + 1 < N_CONV:
                                nmB(i + 1, c)
                                qC.append(c)
                            else:
                                roA(c)
                                qC.append(c)
                        if qA:
                            c = qA.pop(0)
                            if i + 1 < N_CONV:
                                nmA(i + 1, c)
                                qB.append(c)
                            else:
                                qB.append(c)

                    def do_closes():
                        while pend_close:
                            t, dsb = pend_close.pop(0)
                            pt = ps.tile([P, P], F32, space="PSUM", tag="pd",
                                         bufs=2, name=f"tp_{i}_{t}")
                            nc.tensor.transpose(out=pt[:], in_=dsb[:],
                                                identity=ident[:])
                            nc.vector.tensor_add(
                                out=h_fm[:, t * P:(t + 1) * P],
                                in0=h_fm[:, t * P:(t + 1) * P], in1=pt[:])
                            closed_hi[0] = max(closed_hi[0], t + 1)
                        fire_chunks()

                    def s_stage(j):
                        msb = msb_t.pop(j)
                        for a in range(4):
                            s = 4 * j + a
                            for t in tiles_of[s]:
                                if t not in pd_map:
                                    pd_map[t] = ps.tile([P, P], F32, space="PSUM",
                                                        tag="pd", bufs=2,
                                                        name=f"pd_{i}_{t}")
                                nc.tensor.matmul(out=pd_map[t][:],
                                                 lhsT=S_t.pop((s, t))[:],
                                                 rhs=msb[:, a * P:(a + 1) * P],
                                                 start=(s == s_first[t]),
                                                 stop=(s == s_last[t]))
                                if s == s_last[t]:
                                    dsb = sb.tile([P, P], F32, tag="dsb", bufs=4)
                                    nc.vector.tensor_copy(out=dsb[:], in_=pd_map[t][:])
                                    pend_close.append((t, dsb))
                                    del pd_map[t]

                    for jj in range(n512 + LAG + 2):
                        if jj == 0:
                            load_ev(0)
                            if n512 > 1:
                                load_ev(1)
                        if jj + 2 < n512:
                            load_ev(jj + 2)
                        if USE_2HOP:
                            if jj < n512 and jj % EP_TILES == 0:
                                hop1(jj // EP_TILES)
                            if jj < n512:
                                hop2(jj)
                        elif jj < n512:
                            issue_gather(jj)
                        do_closes()
                        if jj < n512:
                            z_stage(jj)
                        if 1 <= jj <= n512:
                            z2_stage(jj - 1)
                        if LAG <= jj < n512 + LAG:
                            p_stage(jj - LAG)
                        if LAG + 1 <= jj < n512 + LAG + 1:
                            m_stage(jj - LAG - 1)
                            s_prep(jj - LAG - 1)
                        if LAG + 2 <= jj < n512 + LAG + 2:
                            s_stage(jj - LAG - 2)
                        pump_chunks()
                    do_closes()
                    while qA or qB or qC:
                        pump_chunks()
                    assert nxt_chunk[0] == N_CHUNK, (i, nxt_chunk[0], closed_hi[0])
                    if i + 1 < N_CONV:
                        issue_cc(i + 1, 2)

    return nc


_CACHE = {}


def _get_runner(meta, reps=1):
    key = (meta, reps)
    if key not in _CACHE:
        nc = build_nc(meta, reps=reps)
        finalize_v2(nc)
        _CACHE[key] = nc
    return _CACHE[key]


def kernel(**inputs):
    core_in, w, meta, (graph_ids, newpos_g) = host_prep(inputs)
    nc = _get_runner(meta)
    in_maps = []
    for c in range(NCORES):
        m = dict(core_in[c])
        m.update(w)
        in_maps.append(m)
    res = run_bass_kernel_spmd(nc, in_maps, core_ids=list(range(NCORES)))
    r_pos = np.concatenate([res.results[c]["r_out"][0, :] for c in range(NCORES)])
    r = r_pos[newpos_g]                         # back to original node order
    out = np.bincount(graph_ids, weights=r.astype(np.float64), minlength=G)[:G]
    return out.astype(np.float32)
